# revision 64
# baseline (speedup 1.0000x reference)
"""T5-style encoder layer (pre-LN, RMSNorm, relative-position bias) on 8 trn2
NeuronCores, data-parallel over the batch dimension (B=8 -> one batch element
per core). Each core runs the full layer for its [S, D] slice; weights and the
relative-bias diagonal blocks are replicated.

Schedule highlights (in-order engines; emission order is the schedule):
- q/k projection GEMMs are interleaved per head-pair into the attention loop
  so the tensor engine computes projections under the exp (ACT) umbrella;
  v is woven into stage 1 (half 0) and the early attention pairs (half 1).
- Attention/exp/AV work in 512-column halves; every PSUM accumulation group
  owns its own 2KB zero-region. AV blocks are emitted one slot late so the
  in-order PE stream never stalls on the exp/bias chain.
- The MLP runs in fp8e4m3 DoubleRow (0.5 PE cycles/row): wi as three terms
  (hi16/lo16 weight split + an fp8 activation-residual correction, ~bf16
  accuracy), wo_mlp as two terms with the 1/16 folded into the PSUM drain.
- wo/rmsnorm/transposes are fused per token block with one-slot lags.

Self-contained: hardcodes all shapes; only depends on the runtime at
/opt/trn_rl_repo.
"""

import sys

if "/opt/trn_rl_repo" not in sys.path:
    sys.path.insert(0, "/opt/trn_rl_repo")

import numpy as np
import ml_dtypes

import concourse.bass as bass
import concourse.tile as tile
from concourse import bacc
from concourse import mybir
from concourse.bass_utils import run_bass_kernel_spmd
from concourse.masks import make_identity

# ---- problem constants -----------------------------------------------------
B, S, D = 8, 1024, 1024
H, HD = 16, 64
MLP = 4096
NUM_BUCKETS, MAX_DIST = 32, 128
EPS = 1e-6
NCORES = 8
P = 128
NS = S // P        # 8 token tiles
ND = D // P        # 8 feature tiles
NM = MLP // P      # 32 mlp tiles
NDIAG = 2 * NS - 1  # 15 distinct 128x128 tile-diagonals of the bias

F32 = mybir.dt.float32
F32R = mybir.dt.float32r
BF16 = mybir.dt.bfloat16
F8 = mybir.dt.float8e4
BF16NP = ml_dtypes.bfloat16
F8NP = ml_dtypes.float8_e4m3fn


def _split8(w):
    """w -> (16*fp8(w), fp8(16*(w - fp8(w)))); PSUM accumulates 16*w."""
    w = np.asarray(w, np.float32)
    hi = w.astype(F8NP)
    hi16 = (hi.astype(np.float32) * 16.0).astype(F8NP)  # exact exponent shift
    lo16 = ((w - hi.astype(np.float32)) * 16.0).astype(F8NP)
    return hi16, lo16


# ---- host-side relative position bias --------------------------------------
def _rel_pos_bucket_np(rel):
    # mirrors t5x _relative_position_bucket (bidirectional), numpy fp32
    n = -rel
    num_buckets = NUM_BUCKETS // 2          # 16
    ret = (n < 0).astype(np.int32) * num_buckets
    n = np.abs(n)
    max_exact = num_buckets // 2            # 8
    is_small = n < max_exact
    val_if_large = max_exact + (
        np.log(n.astype(np.float32) / max_exact + np.finfo(np.float32).eps)
        / np.log(MAX_DIST / max_exact)
        * (num_buckets - max_exact)
    ).astype(np.int32)
    val_if_large = np.minimum(val_if_large, num_buckets - 1)
    return ret + np.where(is_small, n, val_if_large)


def _bias_blocks(rel_emb):
    """[H, 128, NDIAG, 128] f32 blocks of the transposed bias.

    Block d' (=7-m, m = k_tile - q_tile) at [p, c] = bias^T[k, q] for
    k = k_tile*128 + p, q = q_tile*128 + c, i.e. table[1023 + m*128 + p - c].
    """
    rel = np.arange(-(S - 1), S, dtype=np.int32)          # k - q in [-1023, 1023]
    buckets = _rel_pos_bucket_np(rel)                     # [2047]
    table = rel_emb[buckets, :].astype(np.float32)        # [2047, H]
    pp = np.arange(P)[:, None, None]
    dd = np.arange(NDIAG)[None, :, None]
    cc = np.arange(P)[None, None, :]
    idx = 1023 + (NS - 1 - dd) * P + pp - cc              # [128, NDIAG, 128]
    blocks = np.exp(table[idx])                           # [128, NDIAG, 128, H]
    return np.ascontiguousarray(blocks.transpose(3, 0, 1, 2)).astype(BF16NP)


# ---- device kernel ---------------------------------------------------------
def build_nc():
    nc = bacc.Bacc(None, target_bir_lowering=False)

    x_d = nc.declare_dram_parameter("x", [S, D], F32, isOutput=False)
    wq_d = nc.declare_dram_parameter("wq", [D, H * HD], F32R, isOutput=False)
    wk_d = nc.declare_dram_parameter("wk", [D, H * HD], F32R, isOutput=False)
    wv_d = nc.declare_dram_parameter("wv", [D, H * HD], F32R, isOutput=False)
    wo_d = nc.declare_dram_parameter("wo", [H * HD, D], BF16, isOutput=False)
    wih_d = nc.declare_dram_parameter("wih", [D, MLP], F8, isOutput=False)
    wil_d = nc.declare_dram_parameter("wil", [D, MLP], F8, isOutput=False)
    # womlp hi16/lo16 pre-packed as [cpair, partition, (hi0,hi1,lo0,lo1), D]
    wm_d = nc.declare_dram_parameter("womlp8", [NM // 2, P, 4, D], F8, isOutput=False)
    bias_d = nc.declare_dram_parameter("biasb", [H, P, NDIAG, P], BF16, isOutput=False)
    out_d = nc.declare_dram_parameter("out", [S, D], F32, isOutput=True)

    wo_t = wo_d.ap().rearrange("(hp p) d -> p hp d", p=P)
    wq_t = wq_d.ap().rearrange("(di p) m -> p di m", p=P)
    wk_t = wk_d.ap().rearrange("(di p) m -> p di m", p=P)
    wv_t = wv_d.ap().rearrange("(di p) m -> p di m", p=P)
    wih_t = wih_d.ap().rearrange("(di p) m -> p di m", p=P)
    wil_t = wil_d.ap().rearrange("(di p) m -> p di m", p=P)

    with tile.TileContext(nc) as tc:
        _body(nc, tc, x_d, wq_t, wk_t, wv_t, wo_t, wi_t, wm_d,
              bias_d, out_d)
    nc.finalize()
    return nc


def _rmsnorm(nc, pools, src_ap, dst_tile, eps_t):
    """dst = src * rsqrt(mean(src^2) + eps); src [128, D] f32, dst any dtype.

    The square+row-sum is fused on ACT via accum_out (frees DVE and shortens
    the dependency chain).
    """
    var = pools["nrm"].tile([P, 1], F32, tag="var")
    # dst doubles as a dummy sink for the squares; only accum_out is used
    nc.scalar.activation(out=dst_tile, in_=src_ap,
                         func=mybir.ActivationFunctionType.Square,
                         accum_out=var[:, :])
    sd = pools["nrm"].tile([P, 1], F32, tag="sd")
    nc.scalar.activation(out=sd, in_=var, func=mybir.ActivationFunctionType.Sqrt,
                         bias=eps_t[:, :], scale=1.0 / D)
    rstd = pools["nrm"].tile([P, 1], F32, tag="rstd")
    nc.vector.reciprocal(out=rstd, in_=sd)
    nc.scalar.activation(out=dst_tile, in_=src_ap,
                         func=mybir.ActivationFunctionType.Copy,
                         bias=0.0, scale=rstd[:, :])


def _transpose_into(nc, psum_pool, src_tile, dst, si, ident):
    """PE-transpose [128, D] f32/bf16 src into dst[:, di, si*128:...].

    PSUM->SBUF copies ride DVE (ACT is the scarcer engine here).
    """
    for di in range(ND):
        ps = psum_pool.tile([P, P], F32, space="PSUM", tag="tp")
        nc.tensor.transpose(ps[:, :], src_tile[:, di * P:(di + 1) * P], ident[:, :])
        nc.vector.tensor_copy(out=dst[:, di, si * P:(si + 1) * P], in_=ps[:, :])


def _body(nc, tc, x_d, wq_t, wk_t, wv_t, wo_t, wi_t, wm_d, bias_d, out_d):
    fp = {}  # pools

    def pool(name, bufs, space="SBUF"):
        p = tc.alloc_tile_pool(name=name, bufs=bufs, space=space)
        fp[name] = p
        return p

    AF = mybir.ActivationFunctionType
    ALU = mybir.AluOpType

    singles = pool("singles", 1)
    ident32 = singles.tile([P, P], F32)
    make_identity(nc, ident32)
    ident16 = singles.tile([P, P], BF16)
    make_identity(nc, ident16)
    eps_t = singles.tile([P, 1], F32)
    nc.vector.memset(eps_t, EPS)

    pool("sc", 2)      # [128, D] scratch
    pool("nrm", 8)     # [128, 1] norm scalars
    pool("xs", 2)      # x stream tiles

    # activations that live through the attention block; q/k die at the end
    # of the interleaved attention, v/attn_tok live until the fused stage.
    qk_act = tc.alloc_tile_pool(name="qk_act", bufs=1)
    qT = qk_act.tile([P, ND, S], F32R)      # q^T  [hhd, s]
    kT = qk_act.tile([P, ND, S], F32R)      # k^T  [hhd, s]
    va_act = tc.alloc_tile_pool(name="va_act", bufs=1, side="right")
    v_ext = va_act.tile([P, NS, H, HD + 1], BF16)   # [tok, stile, h, hd|1]
    attn_tok = va_act.tile([P, NS, H, HD], BF16)    # normalized attn, token-major

    nc.vector.memset(v_ext[:, :, :, HD:HD + 1], 1.0)

    # ---- stage 1: rmsnorm(x) -> hT (feature-major) -------------------------
    with tc.tile_pool(name="hT_pool", bufs=1) as hT_pool:
        hT = hT_pool.tile([P, ND, S], F32R)
        with tc.tile_pool(name="tp1", bufs=4, space="PSUM") as tp1:
            pending_h = None
            for si in range(NS):
                xt = fp["xs"].tile([P, D], F32, tag="x")
                nc.sync.dma_start(out=xt, in_=x_d.ap()[si * P:(si + 1) * P, :])
                ht = fp["sc"].tile([P, D], F32, tag="h")
                _rmsnorm(nc, fp, xt[:, :], ht, eps_t)
                if pending_h is not None:
                    _transpose_into(nc, tp1, pending_h[1], hT, pending_h[0], ident32)
                pending_h = (si, ht)
            _transpose_into(nc, tp1, pending_h[1], hT, pending_h[0], ident32)

        # ---- stage 2: QKV projections (fp32r) -------------------------------
        with tc.tile_pool(name="wqkv", bufs=3) as wqkv, \
             tc.tile_pool(name="psqkv", bufs=2, space="PSUM") as psqkv, \
             tc.tile_pool(name="psv", bufs=2, space="PSUM") as psv:
            # half-major order so heads 0-7 (q, k, and v) are all ready while
            # half 1 is still streaming -> attention overlaps stage 2.
            for half in range(2):
                for (w_ap, dstT) in ((wq_t, qT), (wk_t, kT)):
                    w_sb = wqkv.tile([P, ND, 512], F32R, tag="w")
                    nc.sync.dma_start(out=w_sb, in_=w_ap[:, :, half * 512:(half + 1) * 512])
                    for mj in range(4):
                        m0 = half * 4 + mj
                        ps = psqkv.tile([P, S], F32, space="PSUM", tag="qkv")
                        for di in range(ND):
                            for sh in range(2):
                                nc.tensor.matmul(
                                    ps[:, sh * 512:(sh + 1) * 512],
                                    w_sb[:, di, mj * P:(mj + 1) * P],
                                    hT[:, di, sh * 512:(sh + 1) * 512],
                                    start=(di == 0), stop=(di == ND - 1),
                                )
                        nc.vector.tensor_copy(out=dstT[:, m0, :], in_=ps[:, :])
                # v: token-major, written into v_ext with the ones column gap
                w_sb = wqkv.tile([P, ND, 512], F32R, tag="w")
                nc.sync.dma_start(out=w_sb, in_=wv_t[:, :, half * 512:(half + 1) * 512])
                for ci in range(NS):
                    ps = psv.tile([P, 512], F32, space="PSUM", tag="vps")
                    for di in range(ND):
                        nc.tensor.matmul(
                            ps[:, :],
                            hT[:, di, ci * P:(ci + 1) * P],
                            w_sb[:, di, :],
                            start=(di == 0), stop=(di == ND - 1),
                        )
                    nc.scalar.copy(
                        out=v_ext[:, ci, half * 8:half * 8 + 8, 0:HD],
                        in_=ps[:, :].rearrange("p (h e) -> p h e", e=HD),
                    )

    # ---- stage 3: attention per head ---------------------------------------
    # AV runs transposed (stationary = wexp block, moving = v column-block of
    # 65) so each [q,k] tile costs 65 moving columns instead of 128. The
    # per-query denominator lands on the partition dim, so normalization is a
    # cheap per-partition scalar multiply (no DRAM-bounce broadcast).
    attnT_pool = tc.alloc_tile_pool(name="attnT_pool", bufs=1)
    # attn^T packed: head 2i on partitions 0-63, head 2i+1 on 64-127
    attnT = attnT_pool.tile([P, H // 2, S], BF16)
    with (
        tc.tile_pool(name="biasp", bufs=2) as biasp,
        tc.tile_pool(name="wexpp", bufs=4) as wexpp,
        tc.tile_pool(name="asb", bufs=4) as asb,
        tc.tile_pool(name="rp", bufs=8) as rp,
        tc.tile_pool(name="lgp", bufs=2, space="PSUM") as lgp,
        tc.tile_pool(name="aup", bufs=1, space="PSUM") as aup,
        tc.tile_pool(name="tp3", bufs=2, space="PSUM") as tp3,
    ):
        au_tiles = {}

        def emit_av(h, ki, wexp):
            # lazy alloc: with bufs=1 the pool slot must not rotate until the
            # previous head's tile has been fully read (emit_extract).
            if h not in au_tiles:
                au_tiles[h] = aup.tile([P, NS, P], F32, tag="au",
                                       name=f"au_{h}")  # [q, qb, hd|den]
            au_t = au_tiles[h]
            for qb in range(NS):
                nc.tensor.matmul(
                    au_t[:, qb, 0:HD + 1],
                    wexp[:, qb * P:(qb + 1) * P],
                    v_ext[:, ki, h, :],
                    start=(ki == 0), stop=(ki == NS - 1),
                )

        def emit_extract(h):
            hb2 = HD * (h % 2)
            au_t = au_tiles.pop(h)
            for qb in range(NS):
                rden = rp.tile([P, 1], F32, tag="rden")
                nc.vector.reciprocal(out=rden, in_=au_t[:, qb, HD:HD + 1])
                a_sb = asb.tile([P, HD], BF16, tag="asb")
                nc.gpsimd.tensor_scalar(out=a_sb, in0=au_t[:, qb, 0:HD],
                                        scalar1=rden[:, :], scalar2=None,
                                        op0=ALU.mult)
                ps = tp3.tile([HD, P], BF16, space="PSUM", tag="tp3")
                nc.tensor.transpose(ps[:, :], a_sb[:, :], ident16[:, :])
                nc.vector.tensor_copy(
                    out=attnT[hb2:hb2 + HD, h // 2, qb * P:(qb + 1) * P], in_=ps[:, :])

        # Flat software pipeline over (h, ki): the AV block for each tile is
        # emitted one slot late (after the next tile's QK matmuls), so the
        # in-order PE stream never waits on the exp/bias chain, including
        # across head boundaries. Extraction of head h is emitted right after
        # its last AV block.
        pending = None
        for h in range(H):
            hb = HD * (h % 2)           # partition base of this head in qT/kT
            hm = h // 2
            bias_sb = biasp.tile([P, NDIAG, P], BF16, tag="bias")
            nc.sync.dma_start(out=bias_sb, in_=bias_d.ap()[h])
            for ki in range(NS):
                lg = lgp.tile([P, S], F32, tag="lg")
                for qh in range(2):
                    nc.tensor.matmul(
                        lg[:, qh * 512:(qh + 1) * 512],
                        kT[hb:hb + HD, hm, ki * P:(ki + 1) * P],
                        qT[hb:hb + HD, hm, qh * 512:(qh + 1) * 512],
                        start=True, stop=True,
                    )
                if pending is not None:
                    emit_av(*pending)
                    if pending[1] == NS - 1:
                        emit_extract(pending[0])
                # w = exp(l) * exp(bias): exp on ACT straight from PSUM,
                # then an all-bf16 SBUF multiply on DVE (2x mode)
                ex = wexpp.tile([P, S], BF16, tag="ex")
                nc.scalar.activation(out=ex, in_=lg[:, :], func=AF.Exp)
                wexp = wexpp.tile([P, S], BF16, tag="wexp")
                nc.vector.tensor_mul(
                    out=wexp[:, :].rearrange("p (c w) -> p c w", w=P),
                    in0=ex[:, :].rearrange("p (c w) -> p c w", w=P),
                    in1=bias_sb[:, NS - 1 - ki:2 * NS - 1 - ki, :],
                )
                pending = (h, ki, wexp)
        emit_av(*pending)
        emit_extract(pending[0])

    # ---- stage 4: attn @ wo + residual -------------------------------------
    qk_act.release()
    out1_pool = tc.alloc_tile_pool(name="out1_pool", bufs=1, side="right")
    out1 = out1_pool.tile([P, NS, D], F32)    # x + attn_out, token-major
    with tc.tile_pool(name="wop", bufs=1) as wop, \
         tc.tile_pool(name="ops", bufs=2, space="PSUM") as ops:
        wo_sb = wop.tile([P, H // 2, D], BF16)
        nc.sync.dma_start(out=wo_sb, in_=wo_t[:, :, :])
        for si in range(NS):
            ps = ops.tile([P, D], F32, tag="wo")
            for hp in range(H // 2):
                for dh in range(2):
                    nc.tensor.matmul(
                        ps[:, dh * 512:(dh + 1) * 512],
                        attnT[:, hp, si * P:(si + 1) * P],
                        wo_sb[:, hp, dh * 512:(dh + 1) * 512],
                        start=(hp == 0), stop=(hp == H // 2 - 1),
                    )
            xt = fp["xs"].tile([P, D], F32, tag="x")
            nc.sync.dma_start(out=xt, in_=x_d.ap()[si * P:(si + 1) * P, :])
            nc.vector.tensor_add(out=out1[:, si, :], in0=ps[:, :], in1=xt[:, :])
    attnT_pool.release()
    qkv_act.release()

    # ---- stage 5: rmsnorm(out1) -> h2T (bf16, feature-major) ---------------
    DR = mybir.MatmulPerfMode.DoubleRow
    with tc.tile_pool(name="h2T_pool", bufs=1, side="right") as h2T_pool, \
         tc.tile_pool(name="yT_pool", bufs=1, side="right") as yT_pool:
        h8T = h2T_pool.tile([P, ND, S], F8)      # fp8(h2^T)
        dh8T = h2T_pool.tile([P, ND, S], F8)     # fp8(h2^T - fp8(h2^T))
        h2Tb = h2T_pool.tile([P, ND, S], BF16)   # full h2^T for the residual
        with tc.tile_pool(name="tp5", bufs=4, space="PSUM") as tp5, \
             tc.tile_pool(name="sc5", bufs=2) as sc5:
            for si in range(NS):
                h2 = sc5.tile([P, D], BF16, tag="h2")
                _rmsnorm(nc, fp, out1[:, si, :], h2, eps_t)
                for di in range(ND):
                    ps = tp5.tile([P, P], BF16, space="PSUM", tag="tp16")
                    nc.tensor.transpose(ps[:, :], h2[:, di * P:(di + 1) * P], ident16[:, :])
                    nc.vector.tensor_copy(out=h2T[:, di, si * P:(si + 1) * P], in_=ps[:, :])

        # ---- stage 6: y^T = relu(wi^T @ h2^T); fp8 DoubleRow, 3 terms -------
        # PSUM accumulates 16*z via h8@(16*fp8(wi)) + h8@fp8(16*dwi) +
        # dh8@(16*fp8(wi)); the relu folds in the 1/16.
        yT = yT_pool.tile([P, NM, S], F8)
        with tc.tile_pool(name="wip", bufs=2) as wip, \
             tc.tile_pool(name="psy", bufs=2, space="PSUM") as psy:
            for eighth in range(8):
                c0 = eighth * (MLP // 8)
                wi_hi = wip.tile([P, ND, MLP // 8], F8, tag="wih")
                nc.sync.dma_start(out=wi_hi, in_=wih_t[:, :, c0:c0 + MLP // 8])
                wi_lo = wip.tile([P, ND, MLP // 8], F8, tag="wil")
                nc.sync.dma_start(out=wi_lo, in_=wil_t[:, :, c0:c0 + MLP // 8])
                for mj in range(NM // 8):
                    m0 = eighth * (NM // 8) + mj
                    ps = psy.tile([P, S], F32, space="PSUM", tag="y")
                    terms = ((wi_hi, h8T), (wi_lo, h8T), (wi_hi, dh8T))
                    for dp in range(ND // 2):
                        for tl, (w_sb, act) in enumerate(terms):
                            for sh in range(2):
                                nc.tensor.matmul(
                                    ps[:, sh * 512:(sh + 1) * 512],
                                    w_sb[:, 2 * dp:2 * dp + 2, mj * P:(mj + 1) * P],
                                    act[:, 2 * dp:2 * dp + 2, sh * 512:(sh + 1) * 512],
                                    start=(dp == 0 and tl == 0),
                                    stop=(dp == ND // 2 - 1 and tl == 2),
                                    perf_mode=DR,
                                )
                    nc.scalar.activation(out=yT[:, m0, :], in_=ps[:, :],
                                         func=AF.Relu, bias=0.0, scale=1.0 / 16.0)

        # ---- stage 7: out = out1 + y^T.T @ womlp; fp8 DoubleRow hi/lo -------
        # Two phases over D halves; 8 concurrent [P, 512] PSUM tiles (one per
        # token block) so each womlp element is read exactly once.
        with tc.tile_pool(name="wmp", bufs=8) as wmp, \
             tc.tile_pool(name="sc7", bufs=2) as sc7, \
             tc.tile_pool(name="o2ps", bufs=8, space="PSUM") as o2ps:
            for ph, (dh, sg) in enumerate(((0, 0), (0, 1), (1, 0), (1, 1))):
                # 4 phases of 4 token blocks: each phase's PSUM drain overlaps
                # the next phase's matmuls (the pool alternates bank halves);
                # womlp is streamed once per (dh, sg) pair.
                dc = dh * 512
                pss = [o2ps.tile([P, 512], F32, tag="o2", name=f"o2_{ph}_{i}")
                       for i in range(4)]
                for cp in range(NM // 2):
                    wmc = wmp.tile([P, 4, 512], F8, tag="wm",
                                   name=f"wm_{ph}_{cp}")
                    nc.sync.dma_start(out=wmc, in_=wm_d.ap()[cp][:, :, dc:dc + 512])
                    for wl in range(2):
                        for i4 in range(4):
                            si = sg * 4 + i4
                            nc.tensor.matmul(
                                pss[i4][:, :],
                                yT[:, 2 * cp:2 * cp + 2, si * P:(si + 1) * P],
                                wmc[:, 2 * wl:2 * wl + 2, :],
                                start=(cp == 0 and wl == 0),
                                stop=(cp == NM // 2 - 1 and wl == 1),
                                perf_mode=DR,
                            )
                for i4 in range(4):
                    si = sg * 4 + i4
                    oo_s = sc7.tile([P, 512], F32, tag="oos")
                    nc.scalar.activation(out=oo_s, in_=pss[i4][:, :],
                                         func=AF.Copy, bias=0.0, scale=1.0 / 16.0)
                    oo = sc7.tile([P, 512], F32, tag="oo")
                    nc.vector.tensor_add(out=oo, in0=oo_s, in1=out1[:, si, dc:dc + 512])
                    nc.sync.dma_start(out=out_d.ap()[si * P:(si + 1) * P, dc:dc + 512], in_=oo)

    out1_pool.release()
    va_act.release()
    for name in ("xs", "nrm", "sc", "singles"):
        fp[name].release()


# ---- host wrapper ----------------------------------------------------------
_NC_CACHE = {}


def _get_nc():
    if "nc" not in _NC_CACHE:
        _NC_CACHE["nc"] = build_nc()
    return _NC_CACHE["nc"]


def _get_exec():
    """Compile once: a sharded PJRT executable over the 8 NeuronCores."""
    if "exec" in _NC_CACHE:
        return _NC_CACHE["exec"]
    import jax
    from jax.sharding import Mesh, PartitionSpec, NamedSharding
    from jax.experimental.shard_map import shard_map
    from concourse.bass2jax import (
        _bass_exec_p, install_neuronx_cc_hook, partition_id_tensor,
    )

    nc = _get_nc()
    install_neuronx_cc_hook()
    pname = nc.partition_id_tensor.name if nc.partition_id_tensor else None
    in_names, out_names, out_avals, zero_outs = [], [], [], []
    for alloc in nc.m.functions[0].allocations:
        if not isinstance(alloc, mybir.MemoryLocationSet):
            continue
        name = alloc.memorylocations[0].name
        if alloc.kind == "ExternalInput":
            if name != pname:
                in_names.append(name)
        elif alloc.kind == "ExternalOutput":
            out_names.append(name)
            shape = tuple(alloc.tensor_shape)
            dtype = mybir.dt.np(alloc.dtype)
            out_avals.append(jax.core.ShapedArray(shape, dtype))
            zero_outs.append(np.zeros(shape, dtype))
    n_params = len(in_names)
    all_in_names = in_names + out_names + ([pname] if pname else [])

    def _body(*args):
        operands = list(args)
        if pname is not None:
            operands.append(partition_id_tensor())
        outs = _bass_exec_p.bind(
            *operands,
            out_avals=tuple(out_avals),
            in_names=tuple(all_in_names),
            out_names=tuple(out_names),
            lowering_input_output_aliases=(),
            sim_require_finite=True,
            sim_require_nnan=True,
            nc=nc,
        )
        return tuple(outs)

    n_outs = len(out_avals)
    devices = jax.devices()[:NCORES]
    mesh = Mesh(np.asarray(devices), ("core",))
    sharded = jax.jit(
        shard_map(_body, mesh=mesh,
                  in_specs=(PartitionSpec("core"),) * (n_params + n_outs),
                  out_specs=(PartitionSpec("core"),) * n_outs,
                  check_rep=False),
        donate_argnums=tuple(range(n_params, n_params + n_outs)),
        keep_unused=True,
    )
    sh = NamedSharding(mesh, PartitionSpec("core"))
    _NC_CACHE["exec"] = (sharded, in_names, out_names, zero_outs, sh)
    return _NC_CACHE["exec"]


def _prep_inputs(x, ln1_scale, wq, wk, wv, wo_attn, ln2_scale, wi, wo_mlp, rel_emb):
    x = np.asarray(x, np.float32)
    ln1 = np.asarray(ln1_scale, np.float32)[:, None]
    ln2 = np.asarray(ln2_scale, np.float32)[:, None]
    wq_h = (np.asarray(wq, np.float32) * ln1).astype(np.float32)
    wk_h = (np.asarray(wk, np.float32) * ln1).astype(np.float32)
    wv_h = (np.asarray(wv, np.float32) * ln1).astype(np.float32)
    wo_h = np.asarray(wo_attn, np.float32).astype(BF16NP)
    wi_hi, wi_lo = _split8(np.asarray(wi, np.float32) * ln2)
    wm_hi, wm_lo = _split8(np.asarray(wo_mlp, np.float32))
    # pack [cpair, partition, (hi ktile0, hi ktile1, lo ktile0, lo ktile1), D]
    hi_v = wm_hi.reshape(NM, P, D)
    lo_v = wm_lo.reshape(NM, P, D)
    wm8 = np.empty((NM // 2, P, 4, D), F8NP)
    wm8[:, :, 0] = hi_v[0::2]
    wm8[:, :, 1] = hi_v[1::2]
    wm8[:, :, 2] = lo_v[0::2]
    wm8[:, :, 3] = lo_v[1::2]
    biasb = _bias_blocks(np.asarray(rel_emb, np.float32))
    shared = {
        "wq": wq_h, "wk": wk_h, "wv": wv_h, "wo": wo_h,
        "wih": wi_hi, "wil": wi_lo, "womlp8": np.ascontiguousarray(wm8),
        "biasb": biasb,
    }
    in_maps = [dict(shared, x=np.ascontiguousarray(x[b])) for b in range(NCORES)]
    return in_maps


def kernel(x, ln1_scale, wq, wk, wv, wo_attn, ln2_scale, wi, wo_mlp, rel_emb):
    import jax
    in_maps = _prep_inputs(x, ln1_scale, wq, wk, wv, wo_attn, ln2_scale,
                           wi, wo_mlp, rel_emb)
    sharded, in_names, out_names, zero_outs, sh = _get_exec()
    concat_in = [
        jax.device_put(
            np.concatenate([in_maps[c][n] for c in range(NCORES)], axis=0), sh)
        for n in in_names
    ]
    czero = [
        jax.device_put(np.zeros((NCORES * z.shape[0], *z.shape[1:]), z.dtype), sh)
        for z in zero_outs
    ]
    outs = sharded(*concat_in, *czero)
    oidx = out_names.index("out")
    full = np.asarray(outs[oidx]).reshape(NCORES, S, D)
    return full.astype(np.float32)



# revision 65
# speedup vs baseline: 1.0073x; 1.0073x over previous
"""T5-style encoder layer (pre-LN, RMSNorm, relative-position bias) on 8 trn2
NeuronCores, data-parallel over the batch dimension (B=8 -> one batch element
per core). Each core runs the full layer for its [S, D] slice; weights and the
relative-bias diagonal blocks are replicated.

Schedule highlights (in-order engines; emission order is the schedule):
- q/k projection GEMMs are interleaved per head-pair into the attention loop
  so the tensor engine computes projections under the exp (ACT) umbrella;
  v is woven into stage 1 (half 0) and the early attention pairs (half 1).
- Attention/exp/AV work in 512-column halves; every PSUM accumulation group
  owns its own 2KB zero-region. AV blocks are emitted one slot late so the
  in-order PE stream never stalls on the exp/bias chain.
- The MLP runs in fp8e4m3 DoubleRow (0.5 PE cycles/row): wi as three terms
  (hi16/lo16 weight split + an fp8 activation-residual correction, ~bf16
  accuracy), wo_mlp as two terms with the 1/16 folded into the PSUM drain.
- wo/rmsnorm/transposes are fused per token block with one-slot lags.

Self-contained: hardcodes all shapes; only depends on the runtime at
/opt/trn_rl_repo.
"""

import sys

if "/opt/trn_rl_repo" not in sys.path:
    sys.path.insert(0, "/opt/trn_rl_repo")

import numpy as np
import ml_dtypes

import concourse.bass as bass
import concourse.tile as tile
from concourse import bacc
from concourse import mybir
from concourse.bass_utils import run_bass_kernel_spmd
from concourse.masks import make_identity

# ---- problem constants -----------------------------------------------------
B, S, D = 8, 1024, 1024
H, HD = 16, 64
MLP = 4096
NUM_BUCKETS, MAX_DIST = 32, 128
EPS = 1e-6
NCORES = 8
P = 128
NS = S // P        # 8 token tiles
ND = D // P        # 8 feature tiles
NM = MLP // P      # 32 mlp tiles
NDIAG = 2 * NS - 1  # 15 distinct 128x128 tile-diagonals of the bias

F32 = mybir.dt.float32
F32R = mybir.dt.float32r
BF16 = mybir.dt.bfloat16
F8 = mybir.dt.float8e4
BF16NP = ml_dtypes.bfloat16
F8NP = ml_dtypes.float8_e4m3fn


def _split8(w):
    """w -> (16*fp8(w), fp8(16*(w - fp8(w)))); PSUM accumulates 16*w."""
    w = np.asarray(w, np.float32)
    hi = w.astype(F8NP)
    hi16 = (hi.astype(np.float32) * 16.0).astype(F8NP)  # exact exponent shift
    lo16 = ((w - hi.astype(np.float32)) * 16.0).astype(F8NP)
    return hi16, lo16


# ---- host-side relative position bias --------------------------------------
def _rel_pos_bucket_np(rel):
    # mirrors t5x _relative_position_bucket (bidirectional), numpy fp32
    n = -rel
    num_buckets = NUM_BUCKETS // 2          # 16
    ret = (n < 0).astype(np.int32) * num_buckets
    n = np.abs(n)
    max_exact = num_buckets // 2            # 8
    is_small = n < max_exact
    val_if_large = max_exact + (
        np.log(n.astype(np.float32) / max_exact + np.finfo(np.float32).eps)
        / np.log(MAX_DIST / max_exact)
        * (num_buckets - max_exact)
    ).astype(np.int32)
    val_if_large = np.minimum(val_if_large, num_buckets - 1)
    return ret + np.where(is_small, n, val_if_large)


def _bias_blocks(rel_emb):
    """[H, 128, NDIAG, 128] f32 blocks of the transposed bias.

    Block d' (=7-m, m = k_tile - q_tile) at [p, c] = bias^T[k, q] for
    k = k_tile*128 + p, q = q_tile*128 + c, i.e. table[1023 + m*128 + p - c].
    """
    rel = np.arange(-(S - 1), S, dtype=np.int32)          # k - q in [-1023, 1023]
    buckets = _rel_pos_bucket_np(rel)                     # [2047]
    table = rel_emb[buckets, :].astype(np.float32)        # [2047, H]
    pp = np.arange(P)[:, None, None]
    dd = np.arange(NDIAG)[None, :, None]
    cc = np.arange(P)[None, None, :]
    idx = 1023 + (NS - 1 - dd) * P + pp - cc              # [128, NDIAG, 128]
    blocks = np.exp(table[idx])                           # [128, NDIAG, 128, H]
    return np.ascontiguousarray(blocks.transpose(3, 0, 1, 2)).astype(BF16NP)


# ---- device kernel ---------------------------------------------------------
def build_nc():
    nc = bacc.Bacc(None, target_bir_lowering=False)

    x_d = nc.declare_dram_parameter("x", [S, D], F32, isOutput=False)
    wq_d = nc.declare_dram_parameter("wq", [D, H * HD], F32R, isOutput=False)
    wk_d = nc.declare_dram_parameter("wk", [D, H * HD], F32R, isOutput=False)
    wv_d = nc.declare_dram_parameter("wv", [D, H * HD], F32R, isOutput=False)
    wo_d = nc.declare_dram_parameter("wo", [H * HD, D], BF16, isOutput=False)
    wih_d = nc.declare_dram_parameter("wih", [D, MLP], F8, isOutput=False)
    wil_d = nc.declare_dram_parameter("wil", [D, MLP], F8, isOutput=False)
    # womlp hi16/lo16 pre-packed as [cpair, partition, (hi0,hi1,lo0,lo1), D]
    wm_d = nc.declare_dram_parameter("womlp8", [NM // 2, P, 4, D], F8, isOutput=False)
    bias_d = nc.declare_dram_parameter("biasb", [H, P, NDIAG, P], BF16, isOutput=False)
    out_d = nc.declare_dram_parameter("out", [S, D], F32, isOutput=True)

    wo_t = wo_d.ap().rearrange("(hp p) d -> p hp d", p=P)
    wq_t = wq_d.ap().rearrange("(di p) m -> p di m", p=P)
    wk_t = wk_d.ap().rearrange("(di p) m -> p di m", p=P)
    wv_t = wv_d.ap().rearrange("(di p) m -> p di m", p=P)
    wih_t = wih_d.ap().rearrange("(di p) m -> p di m", p=P)
    wil_t = wil_d.ap().rearrange("(di p) m -> p di m", p=P)

    with tile.TileContext(nc) as tc:
        _body(nc, tc, x_d, wq_t, wk_t, wv_t, wo_t, wi_t, wm_d,
              bias_d, out_d)
    nc.finalize()
    return nc


def _rmsnorm(nc, pools, src_ap, dst_tile, eps_t):
    """dst = src * rsqrt(mean(src^2) + eps); src [128, D] f32, dst any dtype.

    The square+row-sum is fused on ACT via accum_out (frees DVE and shortens
    the dependency chain).
    """
    var = pools["nrm"].tile([P, 1], F32, tag="var")
    # dst doubles as a dummy sink for the squares; only accum_out is used
    nc.scalar.activation(out=dst_tile, in_=src_ap,
                         func=mybir.ActivationFunctionType.Square,
                         accum_out=var[:, :])
    sd = pools["nrm"].tile([P, 1], F32, tag="sd")
    nc.scalar.activation(out=sd, in_=var, func=mybir.ActivationFunctionType.Sqrt,
                         bias=eps_t[:, :], scale=1.0 / D)
    rstd = pools["nrm"].tile([P, 1], F32, tag="rstd")
    nc.vector.reciprocal(out=rstd, in_=sd)
    nc.scalar.activation(out=dst_tile, in_=src_ap,
                         func=mybir.ActivationFunctionType.Copy,
                         bias=0.0, scale=rstd[:, :])


def _transpose_into(nc, psum_pool, src_tile, dst, si, ident):
    """PE-transpose [128, D] f32/bf16 src into dst[:, di, si*128:...].

    PSUM->SBUF copies ride DVE (ACT is the scarcer engine here).
    """
    for di in range(ND):
        ps = psum_pool.tile([P, P], F32, space="PSUM", tag="tp")
        nc.tensor.transpose(ps[:, :], src_tile[:, di * P:(di + 1) * P], ident[:, :])
        nc.vector.tensor_copy(out=dst[:, di, si * P:(si + 1) * P], in_=ps[:, :])


def _body(nc, tc, x_d, wq_t, wk_t, wv_t, wo_t, wi_t, wm_d, bias_d, out_d):
    fp = {}  # pools

    def pool(name, bufs, space="SBUF"):
        p = tc.alloc_tile_pool(name=name, bufs=bufs, space=space)
        fp[name] = p
        return p

    AF = mybir.ActivationFunctionType
    ALU = mybir.AluOpType

    singles = pool("singles", 1)
    ident32 = singles.tile([P, P], F32)
    make_identity(nc, ident32)
    ident16 = singles.tile([P, P], BF16)
    make_identity(nc, ident16)
    eps_t = singles.tile([P, 1], F32)
    nc.vector.memset(eps_t, EPS)

    pool("sc", 2)      # [128, D] scratch
    pool("nrm", 8)     # [128, 1] norm scalars
    pool("xs", 2)      # x stream tiles

    # activations that live through the attention block; q/k die at the end
    # of the interleaved attention, v/attn_tok live until the fused stage.
    qk_act = tc.alloc_tile_pool(name="qk_act", bufs=1)
    qT = qk_act.tile([P, ND, S], F32R)      # q^T  [hhd, s]
    kT = qk_act.tile([P, ND, S], F32R)      # k^T  [hhd, s]
    va_act = tc.alloc_tile_pool(name="va_act", bufs=1, side="right")
    v_ext = va_act.tile([P, NS, H, HD + 1], BF16)   # [tok, stile, h, hd|1]
    attn_tok = va_act.tile([P, NS, H, HD], BF16)    # normalized attn, token-major

    nc.vector.memset(v_ext[:, :, :, HD:HD + 1], 1.0)

    # ---- stage 1: rmsnorm(x) -> hT (feature-major) -------------------------
    with tc.tile_pool(name="hT_pool", bufs=1) as hT_pool:
        hT = hT_pool.tile([P, ND, S], F32R)
        with tc.tile_pool(name="tp1", bufs=4, space="PSUM") as tp1:
            pending_h = None
            for si in range(NS):
                xt = fp["xs"].tile([P, D], F32, tag="x")
                nc.sync.dma_start(out=xt, in_=x_d.ap()[si * P:(si + 1) * P, :])
                ht = fp["sc"].tile([P, D], F32, tag="h")
                _rmsnorm(nc, fp, xt[:, :], ht, eps_t)
                if pending_h is not None:
                    _transpose_into(nc, tp1, pending_h[1], hT, pending_h[0], ident32)
                pending_h = (si, ht)
            _transpose_into(nc, tp1, pending_h[1], hT, pending_h[0], ident32)

        # ---- stage 2: QKV projections (fp32r) -------------------------------
        with tc.tile_pool(name="wqkv", bufs=3) as wqkv, \
             tc.tile_pool(name="psqkv", bufs=2, space="PSUM") as psqkv, \
             tc.tile_pool(name="psv", bufs=2, space="PSUM") as psv:
            # half-major order so heads 0-7 (q, k, and v) are all ready while
            # half 1 is still streaming -> attention overlaps stage 2.
            for half in range(2):
                for (w_ap, dstT) in ((wq_t, qT), (wk_t, kT)):
                    w_sb = wqkv.tile([P, ND, 512], F32R, tag="w")
                    nc.sync.dma_start(out=w_sb, in_=w_ap[:, :, half * 512:(half + 1) * 512])
                    for mj in range(4):
                        m0 = half * 4 + mj
                        ps = psqkv.tile([P, S], F32, space="PSUM", tag="qkv")
                        for di in range(ND):
                            for sh in range(2):
                                nc.tensor.matmul(
                                    ps[:, sh * 512:(sh + 1) * 512],
                                    w_sb[:, di, mj * P:(mj + 1) * P],
                                    hT[:, di, sh * 512:(sh + 1) * 512],
                                    start=(di == 0), stop=(di == ND - 1),
                                )
                        nc.vector.tensor_copy(out=dstT[:, m0, :], in_=ps[:, :])
                # v: token-major, written into v_ext with the ones column gap
                w_sb = wqkv.tile([P, ND, 512], F32R, tag="w")
                nc.sync.dma_start(out=w_sb, in_=wv_t[:, :, half * 512:(half + 1) * 512])
                for ci in range(NS):
                    ps = psv.tile([P, 512], F32, space="PSUM", tag="vps")
                    for di in range(ND):
                        nc.tensor.matmul(
                            ps[:, :],
                            hT[:, di, ci * P:(ci + 1) * P],
                            w_sb[:, di, :],
                            start=(di == 0), stop=(di == ND - 1),
                        )
                    nc.scalar.copy(
                        out=v_ext[:, ci, half * 8:half * 8 + 8, 0:HD],
                        in_=ps[:, :].rearrange("p (h e) -> p h e", e=HD),
                    )

    # ---- stage 3: attention per head ---------------------------------------
    # AV runs transposed (stationary = wexp block, moving = v column-block of
    # 65) so each [q,k] tile costs 65 moving columns instead of 128. The
    # per-query denominator lands on the partition dim, so normalization is a
    # cheap per-partition scalar multiply (no DRAM-bounce broadcast).
    attnT_pool = tc.alloc_tile_pool(name="attnT_pool", bufs=1)
    # attn^T packed: head 2i on partitions 0-63, head 2i+1 on 64-127
    attnT = attnT_pool.tile([P, H // 2, S], BF16)
    with (
        tc.tile_pool(name="biasp", bufs=2) as biasp,
        tc.tile_pool(name="wexpp", bufs=4) as wexpp,
        tc.tile_pool(name="asb", bufs=4) as asb,
        tc.tile_pool(name="rp", bufs=8) as rp,
        tc.tile_pool(name="lgp", bufs=2, space="PSUM") as lgp,
        tc.tile_pool(name="aup", bufs=1, space="PSUM") as aup,
        tc.tile_pool(name="tp3", bufs=2, space="PSUM") as tp3,
    ):
        au_tiles = {}

        def emit_av(h, ki, wexp):
            # lazy alloc: with bufs=1 the pool slot must not rotate until the
            # previous head's tile has been fully read (emit_extract).
            if h not in au_tiles:
                au_tiles[h] = aup.tile([P, NS, P], F32, tag="au",
                                       name=f"au_{h}")  # [q, qb, hd|den]
            au_t = au_tiles[h]
            for qb in range(NS):
                nc.tensor.matmul(
                    au_t[:, qb, 0:HD + 1],
                    wexp[:, qb * P:(qb + 1) * P],
                    v_ext[:, ki, h, :],
                    start=(ki == 0), stop=(ki == NS - 1),
                )

        def emit_extract(h):
            hb2 = HD * (h % 2)
            au_t = au_tiles.pop(h)
            for qb in range(NS):
                rden = rp.tile([P, 1], F32, tag="rden")
                nc.vector.reciprocal(out=rden, in_=au_t[:, qb, HD:HD + 1])
                a_sb = asb.tile([P, HD], BF16, tag="asb")
                nc.gpsimd.tensor_scalar(out=a_sb, in0=au_t[:, qb, 0:HD],
                                        scalar1=rden[:, :], scalar2=None,
                                        op0=ALU.mult)
                ps = tp3.tile([HD, P], BF16, space="PSUM", tag="tp3")
                nc.tensor.transpose(ps[:, :], a_sb[:, :], ident16[:, :])
                nc.vector.tensor_copy(
                    out=attnT[hb2:hb2 + HD, h // 2, qb * P:(qb + 1) * P], in_=ps[:, :])

        # Flat software pipeline over (h, ki): the AV block for each tile is
        # emitted one slot late (after the next tile's QK matmuls), so the
        # in-order PE stream never waits on the exp/bias chain, including
        # across head boundaries. Extraction of head h is emitted right after
        # its last AV block.
        pending = None
        for h in range(H):
            hb = HD * (h % 2)           # partition base of this head in qT/kT
            hm = h // 2
            bias_sb = biasp.tile([P, NDIAG, P], BF16, tag="bias")
            nc.sync.dma_start(out=bias_sb, in_=bias_d.ap()[h])
            for ki in range(NS):
                lg = lgp.tile([P, S], F32, tag="lg")
                for qh in range(2):
                    nc.tensor.matmul(
                        lg[:, qh * 512:(qh + 1) * 512],
                        kT[hb:hb + HD, hm, ki * P:(ki + 1) * P],
                        qT[hb:hb + HD, hm, qh * 512:(qh + 1) * 512],
                        start=True, stop=True,
                    )
                if pending is not None:
                    emit_av(*pending)
                    if pending[1] == NS - 1:
                        emit_extract(pending[0])
                # w = exp(l) * exp(bias): exp on ACT straight from PSUM,
                # then an all-bf16 SBUF multiply on DVE (2x mode)
                ex = wexpp.tile([P, S], BF16, tag="ex")
                nc.scalar.activation(out=ex, in_=lg[:, :], func=AF.Exp)
                wexp = wexpp.tile([P, S], BF16, tag="wexp")
                nc.vector.tensor_mul(
                    out=wexp[:, :].rearrange("p (c w) -> p c w", w=P),
                    in0=ex[:, :].rearrange("p (c w) -> p c w", w=P),
                    in1=bias_sb[:, NS - 1 - ki:2 * NS - 1 - ki, :],
                )
                pending = (h, ki, wexp)
        emit_av(*pending)
        emit_extract(pending[0])

    # ---- stage 4: attn @ wo + residual -------------------------------------
    qk_act.release()
    out1_pool = tc.alloc_tile_pool(name="out1_pool", bufs=1, side="right")
    out1 = out1_pool.tile([P, NS, D], F32)    # x + attn_out, token-major
    with tc.tile_pool(name="wop", bufs=1) as wop, \
         tc.tile_pool(name="ops", bufs=2, space="PSUM") as ops:
        wo_sb = wop.tile([P, H // 2, D], BF16)
        nc.sync.dma_start(out=wo_sb, in_=wo_t[:, :, :])
        for si in range(NS):
            ps = ops.tile([P, D], F32, tag="wo")
            for hp in range(H // 2):
                for dh in range(2):
                    nc.tensor.matmul(
                        ps[:, dh * 512:(dh + 1) * 512],
                        attnT[:, hp, si * P:(si + 1) * P],
                        wo_sb[:, hp, dh * 512:(dh + 1) * 512],
                        start=(hp == 0), stop=(hp == H // 2 - 1),
                    )
            xt = fp["xs"].tile([P, D], F32, tag="x")
            nc.sync.dma_start(out=xt, in_=x_d.ap()[si * P:(si + 1) * P, :])
            nc.vector.tensor_add(out=out1[:, si, :], in0=ps[:, :], in1=xt[:, :])
    attnT_pool.release()
    qkv_act.release()

    # ---- stage 5: rmsnorm(out1) -> h2T (bf16, feature-major) ---------------
    DR = mybir.MatmulPerfMode.DoubleRow
    with tc.tile_pool(name="h2T_pool", bufs=1, side="right") as h2T_pool, \
         tc.tile_pool(name="yT_pool", bufs=1, side="right") as yT_pool:
        h8T = h2T_pool.tile([P, ND, S], F8)      # fp8(h2^T)
        dh8T = h2T_pool.tile([P, ND, S], F8)     # fp8(h2^T - fp8(h2^T))
        h2Tb = h2T_pool.tile([P, ND, S], BF16)   # full h2^T for the residual
        with tc.tile_pool(name="tp5", bufs=4, space="PSUM") as tp5, \
             tc.tile_pool(name="sc5", bufs=2) as sc5:
            for si in range(NS):
                h2 = sc5.tile([P, D], BF16, tag="h2")
                _rmsnorm(nc, fp, out1[:, si, :], h2, eps_t)
                for di in range(ND):
                    ps = tp5.tile([P, P], BF16, space="PSUM", tag="tp16")
                    nc.tensor.transpose(ps[:, :], h2[:, di * P:(di + 1) * P], ident16[:, :])
                    nc.vector.tensor_copy(out=h2T[:, di, si * P:(si + 1) * P], in_=ps[:, :])

        # ---- stage 6: y^T = relu(wi^T @ h2^T); fp8 DoubleRow, 3 terms -------
        # PSUM accumulates 16*z via h8@(16*fp8(wi)) + h8@fp8(16*dwi) +
        # dh8@(16*fp8(wi)); the relu folds in the 1/16.
        yT = yT_pool.tile([P, NM, S], F8)
        with tc.tile_pool(name="wip", bufs=2) as wip, \
             tc.tile_pool(name="psy", bufs=2, space="PSUM") as psy:
            for eighth in range(8):
                c0 = eighth * (MLP // 8)
                wi_hi = wip.tile([P, ND, MLP // 8], F8, tag="wih")
                nc.sync.dma_start(out=wi_hi, in_=wih_t[:, :, c0:c0 + MLP // 8])
                wi_lo = wip.tile([P, ND, MLP // 8], F8, tag="wil")
                nc.sync.dma_start(out=wi_lo, in_=wil_t[:, :, c0:c0 + MLP // 8])
                for mj in range(NM // 8):
                    m0 = eighth * (NM // 8) + mj
                    ps = psy.tile([P, S], F32, space="PSUM", tag="y")
                    terms = ((wi_hi, h8T), (wi_lo, h8T), (wi_hi, dh8T))
                    for dp in range(ND // 2):
                        for tl, (w_sb, act) in enumerate(terms):
                            for sh in range(2):
                                nc.tensor.matmul(
                                    ps[:, sh * 512:(sh + 1) * 512],
                                    w_sb[:, 2 * dp:2 * dp + 2, mj * P:(mj + 1) * P],
                                    act[:, 2 * dp:2 * dp + 2, sh * 512:(sh + 1) * 512],
                                    start=(dp == 0 and tl == 0),
                                    stop=(dp == ND // 2 - 1 and tl == 2),
                                    perf_mode=DR,
                                )
                    nc.scalar.activation(out=yT[:, m0, :], in_=ps[:, :],
                                         func=AF.Relu, bias=0.0, scale=1.0 / 16.0)

        # ---- stage 7: out = out1 + y^T.T @ womlp; fp8 DoubleRow hi/lo -------
        # Two phases over D halves; 8 concurrent [P, 512] PSUM tiles (one per
        # token block) so each womlp element is read exactly once.
        with tc.tile_pool(name="wmp", bufs=10) as wmp, \
             tc.tile_pool(name="sc7", bufs=4) as sc7, \
             tc.tile_pool(name="o2ps", bufs=8, space="PSUM") as o2ps:
            for ph, (dh, sg) in enumerate(((0, 0), (0, 1), (1, 0), (1, 1))):
                # 4 phases of 4 token blocks: each phase's PSUM drain overlaps
                # the next phase's matmuls (the pool alternates bank halves);
                # womlp is streamed once per (dh, sg) pair.
                dc = dh * 512
                pss = [o2ps.tile([P, 512], F32, tag="o2", name=f"o2_{ph}_{i}")
                       for i in range(4)]
                for cp in range(NM // 2):
                    wmc = wmp.tile([P, 4, 512], F8, tag="wm",
                                   name=f"wm_{ph}_{cp}")
                    nc.sync.dma_start(out=wmc, in_=wm_d.ap()[cp][:, :, dc:dc + 512])
                    for wl in range(2):
                        for i4 in range(4):
                            si = sg * 4 + i4
                            nc.tensor.matmul(
                                pss[i4][:, :],
                                yT[:, 2 * cp:2 * cp + 2, si * P:(si + 1) * P],
                                wmc[:, 2 * wl:2 * wl + 2, :],
                                start=(cp == 0 and wl == 0),
                                stop=(cp == NM // 2 - 1 and wl == 1),
                                perf_mode=DR,
                            )
                for i4 in range(4):
                    si = sg * 4 + i4
                    oo_s = sc7.tile([P, 512], F32, tag="oos")
                    nc.scalar.activation(out=oo_s, in_=pss[i4][:, :],
                                         func=AF.Copy, bias=0.0, scale=1.0 / 16.0)
                    oo = sc7.tile([P, 512], F32, tag="oo")
                    nc.vector.tensor_add(out=oo, in0=oo_s, in1=out1[:, si, dc:dc + 512])
                    nc.sync.dma_start(out=out_d.ap()[si * P:(si + 1) * P, dc:dc + 512], in_=oo)

    out1_pool.release()
    va_act.release()
    for name in ("xs", "nrm", "sc", "singles"):
        fp[name].release()


# ---- host wrapper ----------------------------------------------------------
_NC_CACHE = {}


def _get_nc():
    if "nc" not in _NC_CACHE:
        _NC_CACHE["nc"] = build_nc()
    return _NC_CACHE["nc"]


def _get_exec():
    """Compile once: a sharded PJRT executable over the 8 NeuronCores."""
    if "exec" in _NC_CACHE:
        return _NC_CACHE["exec"]
    import jax
    from jax.sharding import Mesh, PartitionSpec, NamedSharding
    from jax.experimental.shard_map import shard_map
    from concourse.bass2jax import (
        _bass_exec_p, install_neuronx_cc_hook, partition_id_tensor,
    )

    nc = _get_nc()
    install_neuronx_cc_hook()
    pname = nc.partition_id_tensor.name if nc.partition_id_tensor else None
    in_names, out_names, out_avals, zero_outs = [], [], [], []
    for alloc in nc.m.functions[0].allocations:
        if not isinstance(alloc, mybir.MemoryLocationSet):
            continue
        name = alloc.memorylocations[0].name
        if alloc.kind == "ExternalInput":
            if name != pname:
                in_names.append(name)
        elif alloc.kind == "ExternalOutput":
            out_names.append(name)
            shape = tuple(alloc.tensor_shape)
            dtype = mybir.dt.np(alloc.dtype)
            out_avals.append(jax.core.ShapedArray(shape, dtype))
            zero_outs.append(np.zeros(shape, dtype))
    n_params = len(in_names)
    all_in_names = in_names + out_names + ([pname] if pname else [])

    def _body(*args):
        operands = list(args)
        if pname is not None:
            operands.append(partition_id_tensor())
        outs = _bass_exec_p.bind(
            *operands,
            out_avals=tuple(out_avals),
            in_names=tuple(all_in_names),
            out_names=tuple(out_names),
            lowering_input_output_aliases=(),
            sim_require_finite=True,
            sim_require_nnan=True,
            nc=nc,
        )
        return tuple(outs)

    n_outs = len(out_avals)
    devices = jax.devices()[:NCORES]
    mesh = Mesh(np.asarray(devices), ("core",))
    sharded = jax.jit(
        shard_map(_body, mesh=mesh,
                  in_specs=(PartitionSpec("core"),) * (n_params + n_outs),
                  out_specs=(PartitionSpec("core"),) * n_outs,
                  check_rep=False),
        donate_argnums=tuple(range(n_params, n_params + n_outs)),
        keep_unused=True,
    )
    sh = NamedSharding(mesh, PartitionSpec("core"))
    _NC_CACHE["exec"] = (sharded, in_names, out_names, zero_outs, sh)
    return _NC_CACHE["exec"]


def _prep_inputs(x, ln1_scale, wq, wk, wv, wo_attn, ln2_scale, wi, wo_mlp, rel_emb):
    x = np.asarray(x, np.float32)
    ln1 = np.asarray(ln1_scale, np.float32)[:, None]
    ln2 = np.asarray(ln2_scale, np.float32)[:, None]
    wq_h = (np.asarray(wq, np.float32) * ln1).astype(np.float32)
    wk_h = (np.asarray(wk, np.float32) * ln1).astype(np.float32)
    wv_h = (np.asarray(wv, np.float32) * ln1).astype(np.float32)
    wo_h = np.asarray(wo_attn, np.float32).astype(BF16NP)
    wi_hi, wi_lo = _split8(np.asarray(wi, np.float32) * ln2)
    wm_hi, wm_lo = _split8(np.asarray(wo_mlp, np.float32))
    # pack [cpair, partition, (hi ktile0, hi ktile1, lo ktile0, lo ktile1), D]
    hi_v = wm_hi.reshape(NM, P, D)
    lo_v = wm_lo.reshape(NM, P, D)
    wm8 = np.empty((NM // 2, P, 4, D), F8NP)
    wm8[:, :, 0] = hi_v[0::2]
    wm8[:, :, 1] = hi_v[1::2]
    wm8[:, :, 2] = lo_v[0::2]
    wm8[:, :, 3] = lo_v[1::2]
    biasb = _bias_blocks(np.asarray(rel_emb, np.float32))
    shared = {
        "wq": wq_h, "wk": wk_h, "wv": wv_h, "wo": wo_h,
        "wih": wi_hi, "wil": wi_lo, "womlp8": np.ascontiguousarray(wm8),
        "biasb": biasb,
    }
    in_maps = [dict(shared, x=np.ascontiguousarray(x[b])) for b in range(NCORES)]
    return in_maps


def kernel(x, ln1_scale, wq, wk, wv, wo_attn, ln2_scale, wi, wo_mlp, rel_emb):
    import jax
    in_maps = _prep_inputs(x, ln1_scale, wq, wk, wv, wo_attn, ln2_scale,
                           wi, wo_mlp, rel_emb)
    sharded, in_names, out_names, zero_outs, sh = _get_exec()
    concat_in = [
        jax.device_put(
            np.concatenate([in_maps[c][n] for c in range(NCORES)], axis=0), sh)
        for n in in_names
    ]
    czero = [
        jax.device_put(np.zeros((NCORES * z.shape[0], *z.shape[1:]), z.dtype), sh)
        for z in zero_outs
    ]
    outs = sharded(*concat_in, *czero)
    oidx = out_names.index("out")
    full = np.asarray(outs[oidx]).reshape(NCORES, S, D)
    return full.astype(np.float32)

